# revision 11
# baseline (speedup 1.0000x reference)
"""DA-RNN style encoder (LSTM + input attention) on 8 Trainium2 cores.

Problem: nn_Encoder_63024350101963
  B=2048, T-1=31 steps, D=128 input feats, H=128 hidden.

Key algebraic fact exploited: in the reference,
    score = (h @ w_h + c @ w_c + b)[:, None] + x_score
the recurrent term is constant along the softmax axis, and softmax is
shift-invariant, so
    attn = softmax(x_score)      (time-constant, recurrence-independent)
Therefore weighted = attn[:,None,:] * x  is a pure elementwise op and only
the LSTM cell recurrence is serial.

Design notes (v3):
  - fp16 end-to-end (tolerance gate 2e-2): halves DMA bytes, all matmuls
    run at 1 cycle/row, DVE gets 2x packed modes.
  - measured HW overhead is dominated by per-instruction dispatch (~150ns
    each on top of the cost model), so the kernel is built to MINIMIZE
    INSTRUCTION COUNT: single 256-wide batch chain, merged sigmoid over
    all four gate chunks, x_score as 31 N=256 matmuls in the transposed
    layout with a matmul-based partition sum + gpsimd partition_broadcast
    for the softmax normalization (no per-subtile loops, no transposes),
    broadcast-AP elementwise ops for the scaled identities and the
    attention-weighted input.
  - LSTM state kept as cc = 2*c so tanh comes from the same single
    sigmoid pass (g gate pre-scaled 2x in the weights).
  - next group's bias/W_ih matmuls are emitted after the current step's
    W_hh + sigmoid so the PE works ahead on them during the pointwise
    latency window.

Sharding: data-parallel over batch, 8 cores x 256 rows, weights replicated.
"""

import numpy as np

T = 31          # time steps (T_ref - 1)
D = 128         # input feature dim
H = 128         # hidden dim
G = 4 * H       # gate rows
NCORES = 8
B = 2048
BL = B // NCORES  # 256 batch rows per core

_CACHE = {}


def _build_program(loop_n=0):
    from contextlib import ExitStack

    import concourse.bacc as bacc
    import concourse.mybir as mybir
    import concourse.tile as tile

    f16 = mybir.dt.float16
    f32 = mybir.dt.float32

    nc = bacc.Bacc("TRN2", target_bir_lowering=False, debug=False)

    xt_d = nc.dram_tensor("xt", [D, T, BL], f16, kind="ExternalInput").ap()
    wxb_d = nc.dram_tensor("wxb", [D, T], f16, kind="ExternalInput").ap()
    wih_d = nc.dram_tensor("wih", [D, G], f16, kind="ExternalInput").ap()
    whh_d = nc.dram_tensor("whh", [H, G], f16, kind="ExternalInput").ap()
    bias_d = nc.dram_tensor("bias", [2, G], f16, kind="ExternalInput").ap()
    ident_d = nc.dram_tensor("ident", [D, D], f16, kind="ExternalInput").ap()
    ones_d = nc.dram_tensor("ones", [2, 2 * BL], f16, kind="ExternalInput").ap()

    wt_d = nc.dram_tensor("wt_out", [T, D, BL], f16, kind="ExternalOutput").ap()
    enc_d = nc.dram_tensor("enc_out", [T, H, BL], f16, kind="ExternalOutput").ap()

    with ExitStack() as ctx:
        tc = ctx.enter_context(tile.TileContext(nc))

        def body():
            _emit(nc, tc, ctx, mybir, f16, f32,
                  xt_d, wxb_d, wih_d, whh_d, bias_d, ident_d, ones_d,
                  wt_d, enc_d)

        if loop_n:
            with tc.For_i(0, loop_n, 1):
                body()
        else:
            body()

    nc.compile()
    return nc


def _emit(nc, tc, ctx, mybir, f16, f32,
          xt_d, wxb_d, wih_d, whh_d, bias_d, ident_d, ones_d, wt_d, enc_d):
    from contextlib import ExitStack

    AF = mybir.ActivationFunctionType

    big = ctx.enter_context(tc.tile_pool(name="big", bufs=1))

    # ---- persistent SBUF tensors ----
    xt_s = big.tile([D, T * BL], f16, tag="xt")
    wxt_s = big.tile([D, T * BL], f16, tag="wxt")
    wid_s = big.tile([D, T * D], f16, tag="wid")
    wxb_s = big.tile([D, T], f16, tag="wxb")
    wih_s = big.tile([D, G], f16, tag="wih")
    whh_s = big.tile([H, G], f16, tag="whh")
    bias_s = big.tile([2, G], f16, tag="bias")
    ident_s = big.tile([D, D], f16, tag="ident")
    ones_s = big.tile([2, 2 * BL], f16, tag="ones")
    zro_s = big.tile([H, BL], f16, tag="zro")
    attnT = big.tile([D, BL], f16, tag="attnT")

    nc.sync.dma_start(out=ident_s[:], in_=ident_d[:])
    nc.sync.dma_start(out=wxb_s[:], in_=wxb_d[:])
    # wid[:, t, :] = w_x[t] * I in ONE broadcast-AP op
    nc.vector.tensor_mul(
        wid_s[:].rearrange("p (t j) -> p t j", t=T),
        ident_s[:].unsqueeze(1).broadcast_to([D, T, D]),
        wxb_s[:].unsqueeze(2).broadcast_to([D, T, D]),
    )
    nc.vector.memset(zro_s[:], 0.0)

    for t0 in range(0, T, 16):
        t1 = min(t0 + 16, T)
        nc.sync.dma_start(
            out=xt_s[:, t0 * BL:t1 * BL], in_=xt_d[:, t0:t1, :])
    nc.sync.dma_start(out=wih_s[:], in_=wih_d[:])
    nc.sync.dma_start(out=whh_s[:], in_=whh_d[:])
    nc.sync.dma_start(out=bias_s[:], in_=bias_d[:])
    nc.sync.dma_start(out=ones_s[:], in_=ones_d[:])

    with ExitStack() as fctx:
        frs = fctx.enter_context(tc.tile_pool(name="fsmall", bufs=2))
        psf = fctx.enter_context(tc.tile_pool(name="psf", bufs=2, space="PSUM"))

        # ---- PE warmup: long dummy matmuls keep the clock ramping while
        # the x DMA lands (output discarded).
        pwm = psf.tile([D, 4 * D], f32, tag="warm")
        for w in range(14):
            nc.tensor.matmul(
                pwm[:], lhsT=ident_s[:],
                rhs=ident_s[:].unsqueeze(1).broadcast_to([D, 4, D]),
                start=True, stop=True, skip_group_check=True,
            )

        # ---- x_score in transposed layout: xsT[d, b] += wid_t.T @ xt_t
        ps_xs = psf.tile([D, BL], f32, tag="xs")
        for t in range(T):
            nc.tensor.matmul(
                ps_xs[:],
                lhsT=wid_s[:, t * D:(t + 1) * D],
                rhs=xt_s[:, t * BL:(t + 1) * BL],
                start=(t == 0),
                stop=(t == T - 1),
                skip_group_check=True,
            )

        # ---- softmax over d (the partition axis), max-free: |score| <~ 3
        # exp -> matmul-sum over partitions -> reciprocal -> partition
        # broadcast -> normalize.  attn = exp(s) / sum_d exp(s)
        exps = frs.tile([D, BL], f16, tag="exps")
        nc.scalar.activation(exps[:], ps_xs[:], AF.Exp)
        one1 = frs.tile([D, 1], f16, tag="one1")
        nc.vector.memset(one1[:], 1.0)
        zsum = psf.tile([1, BL], f32, tag="zsum")
        nc.tensor.matmul(zsum[:], lhsT=one1[:], rhs=exps[:],
                         start=True, stop=True, skip_group_check=True)
        rz = frs.tile([1, BL], f32, tag="rz")
        nc.vector.reciprocal(rz[:], zsum[:])
        rzb = frs.tile([D, BL], f32, tag="rzb")
        nc.gpsimd.partition_broadcast(rzb[:], rz[:])
        nc.vector.tensor_mul(attnT[:], exps[:], rzb[:])

        # pre-trigger the Sigmoid table-set load while the front finishes
        sdum = frs.tile([1, 1], f32, tag="sdum")
        nc.scalar.activation(sdum[:], rz[:, 0:1], AF.Sigmoid)

    # ---- LSTM recurrence (single 256-wide chain) ----
    # PSUM ping-pong tiles [128, 2048] (4 banks); bank c = gate chunk c
    # (pytorch order i,f,g,o; g pre-scaled 2x), cols = [t | t+1].
    psg = ctx.enter_context(tc.tile_pool(name="psg", bufs=2, space="PSUM"))
    sgp = ctx.enter_context(tc.tile_pool(name="sg", bufs=4))
    sm = ctx.enter_context(tc.tile_pool(name="small", bufs=6))
    ccp = ctx.enter_context(tc.tile_pool(name="ccp", bufs=4))
    hst = ctx.enter_context(tc.tile_pool(name="hstage", bufs=3))
    jk = ctx.enter_context(tc.tile_pool(name="junk", bufs=4))

    def emit_group_mms(ps, tg):
        """bias + W_ih matmuls for the 2-step group starting at tg."""
        gw = min(2, T - tg)
        nw = gw * BL
        for c in range(4):
            gseg = slice(c * H, (c + 1) * H)
            nc.tensor.matmul(
                ps[:, c * 512:c * 512 + nw], lhsT=bias_s[0:2, gseg],
                rhs=ones_s[0:2, 0:nw], start=True, stop=False,
                skip_group_check=True,
            )
            nc.tensor.matmul(
                ps[:, c * 512:c * 512 + nw], lhsT=wih_s[:, gseg],
                rhs=wxt_s[:, tg * BL:tg * BL + nw], start=False, stop=False,
                skip_group_check=True,
            )

    def emit_wt(tg):
        """weighted input for a 2-step group in one broadcast-AP op."""
        gw = min(2, T - tg)
        nc.vector.tensor_mul(
            wxt_s[:, tg * BL:(tg + gw) * BL].rearrange(
                "d (t b) -> d t b", t=gw),
            xt_s[:, tg * BL:(tg + gw) * BL].rearrange(
                "d (t b) -> d t b", t=gw),
            attnT[:].unsqueeze(1).broadcast_to([D, gw, BL]),
        )

    c_prev = zro_s
    h_prev = zro_s
    hstage = None
    ps_cur = None
    ps_next = None

    # group 0 prep happens up front (overlaps softmax tail / table load)
    emit_wt(0)
    ps_cur = psg.tile([128, 4 * 512], f32, tag="gates", name="ps_g0")
    emit_group_mms(ps_cur, 0)

    for tg in range(0, T, 2):
        g = tg // 2
        gw = min(2, T - tg)
        if tg + 2 < T:
            emit_wt(tg + 2)
        if tg % 8 == 6 or tg + gw == T:  # flush wt_out every 8 steps
            t0 = (tg // 8) * 8
            t1 = min(t0 + 8, T)
            nc.sync.dma_start(
                out=wt_d[t0:t1].rearrange("t d b -> d t b"),
                in_=wxt_s[:, t0 * BL:t1 * BL].rearrange(
                    "d (t b) -> d t b", b=BL),
            )
        ps_next = None
        for dtw in range(gw):
            t = tg + dtw
            if t % 4 == 0:
                hstage = hst.tile([H, 4 * BL], f16, tag="hst")
            for c in range(4):
                nc.tensor.matmul(
                    ps_cur[:, c * 512 + dtw * BL:c * 512 + (dtw + 1) * BL],
                    lhsT=whh_s[:, c * H:(c + 1) * H],
                    rhs=h_prev[:],
                    start=False, stop=(dtw == gw - 1),
                    skip_group_check=True,
                )
            sg = sgp.tile([128, 4 * BL], f16, tag="sg", name=f"sg_{t}")
            ps_slot = ps_cur[:].rearrange("p (c x) -> p c x", c=4)[
                :, :, dtw * BL:(dtw + 1) * BL]
            nc.scalar.activation(sg[:], ps_slot, AF.Sigmoid)
            # next group's bias/W_ih matmuls: PE works ahead on these
            # during the sigmoid/pointwise latency window.
            if tg + 2 < T and dtw == 0:
                ps_next = psg.tile([128, 4 * 512], f32, tag="gates",
                                   name=f"ps_g{g + 1}")
                emit_group_mms(ps_next, tg + 2)

            si = sg[:, 0 * BL:1 * BL]
            sf = sg[:, 1 * BL:2 * BL]
            s2g = sg[:, 2 * BL:3 * BL]
            so = sg[:, 3 * BL:4 * BL]

            t1_ = sm.tile([H, BL], f16, tag="t1", name=f"t1_{t}")
            nc.gpsimd.tensor_mul(t1_[:], sf, c_prev[:])
            t2 = sm.tile([H, BL], f16, tag="t2", name=f"t2_{t}")
            j1 = jk.tile([H, 1], f32, tag="j1", name=f"j1_{t}")
            # t2 = 2*sigmoid(i)*tanh(g) = (4*s2g - 2) * si
            nc.vector.affine_mul_reduce(
                out=t2[:], accum_out=j1[:], in0=s2g, in1=si,
                scale=4.0, bias=-2.0,
            )
            cc = ccp.tile([H, BL], f16, tag="c", name=f"c_{t}")
            nc.vector.tensor_add(cc[:], t1_[:], t2[:])
            s2c = sm.tile([H, BL], f16, tag="s2c", name=f"s2c_{t}")
            nc.scalar.activation(s2c[:], cc[:], AF.Sigmoid)
            h_new = hstage[:, (t % 4) * BL:(t % 4 + 1) * BL]
            j2 = jk.tile([H, 1], f32, tag="j2", name=f"j2_{t}")
            # h = tanh(c) * sigmoid(o) = (2*s2c - 1) * so
            nc.vector.affine_mul_reduce(
                out=h_new, accum_out=j2[:], in0=s2c[:], in1=so,
                scale=2.0, bias=-1.0,
            )
            c_prev = cc
            h_prev = _Slice(h_new)
            if t % 4 == 3 or t == T - 1:
                t0 = (t // 4) * 4
                n = t - t0 + 1
                nc.sync.dma_start(
                    out=enc_d[t0:t0 + n].rearrange("t h b -> h t b"),
                    in_=hstage[:].rearrange("h (t b) -> h t b", t=4)[:, :n, :],
                )
        ps_cur = ps_next


class _Slice:
    """Tiny adapter so h_prev[:] works for both tiles and AP slices."""

    def __init__(self, ap):
        self._ap = ap

    def __getitem__(self, key):
        return self._ap


def _get_program():
    if "nc" not in _CACHE:
        _CACHE["nc"] = _build_program()
    return _CACHE["nc"]


def _host_inputs(input_data, W_ih, W_hh, b_ih, b_hh, attn_w, attn_b):
    """Build the per-core input maps (host-side prep is weights-only +
    layout transforms + fp16 casts)."""
    x = np.asarray(input_data, dtype=np.float32)
    W_ih = np.asarray(W_ih, dtype=np.float32)
    W_hh = np.asarray(W_hh, dtype=np.float32)
    b = (np.asarray(b_ih, dtype=np.float32)
         + np.asarray(b_hh, dtype=np.float32))
    w_x = np.asarray(attn_w, dtype=np.float32)[2 * H:]  # only the x part

    # scale the g-gate block (pytorch order i,f,g,o -> rows 2H:3H) by 2
    # so tanh(g) = 2*sigmoid(2g) - 1 works with a single sigmoid pass.
    scale = np.ones((G, 1), np.float32)
    scale[2 * H:3 * H] = 2.0
    wih_t = np.ascontiguousarray((W_ih * scale).T).astype(np.float16)
    whh_t = np.ascontiguousarray((W_hh * scale).T).astype(np.float16)
    bm = (b[None, :] * scale.T).astype(np.float32)
    b_hi = bm.astype(np.float16)
    b_lo = (bm - b_hi.astype(np.float32)).astype(np.float16)
    bias_m = np.ascontiguousarray(np.concatenate([b_hi, b_lo], 0))  # [2, 4H]

    wxb = np.ascontiguousarray(
        np.tile(w_x[None, :], (D, 1))).astype(np.float16)  # [D, T]
    ident = np.eye(D, dtype=np.float16)
    ones = np.ones((2, 2 * BL), np.float16)

    in_maps = []
    for i in range(NCORES):
        xs = x[i * BL:(i + 1) * BL]                  # [BL, T, D]
        xt = np.ascontiguousarray(
            xs.transpose(2, 1, 0)).astype(np.float16)  # [D, T, BL]
        in_maps.append({
            "xt": xt,
            "wxb": wxb,
            "wih": wih_t,
            "whh": whh_t,
            "bias": bias_m,
            "ident": ident,
            "ones": ones,
        })
    return in_maps


def _gather(results):
    weighted = np.empty((B, T, D), np.float32)
    encoded = np.empty((B, T, H), np.float32)
    for i, r in enumerate(results):
        # wt_out/enc_out are fp16 [T, D|H, BL] -> [BL, T, D|H] fp32
        weighted[i * BL:(i + 1) * BL] = \
            r["wt_out"].transpose(2, 0, 1).astype(np.float32)
        encoded[i * BL:(i + 1) * BL] = \
            r["enc_out"].transpose(2, 0, 1).astype(np.float32)
    return weighted, encoded


def kernel(input_data, W_ih, W_hh, b_ih, b_hh, attn_w, attn_b):
    from concourse.bass_utils import run_bass_kernel_spmd

    nc = _get_program()
    in_maps = _host_inputs(input_data, W_ih, W_hh, b_ih, b_hh, attn_w, attn_b)
    res = run_bass_kernel_spmd(nc, in_maps, list(range(NCORES)))
    return _gather(res.results)


# revision 16
# speedup vs baseline: 1.2331x; 1.2331x over previous
"""DA-RNN style encoder (LSTM + input attention) on 8 Trainium2 cores.

Problem: nn_Encoder_63024350101963
  B=2048, T-1=31 steps, D=128 input feats, H=128 hidden.

Key algebraic fact exploited: in the reference,
    score = (h @ w_h + c @ w_c + b)[:, None] + x_score
the recurrent term is constant along the softmax axis, and softmax is
shift-invariant, so
    attn = softmax(x_score)      (time-constant, recurrence-independent)
Therefore weighted = attn[:,None,:] * x  is a pure elementwise op and only
the LSTM cell recurrence is serial.

Design notes (v3):
  - fp16 end-to-end (tolerance gate 2e-2): halves DMA bytes, all matmuls
    run at 1 cycle/row, DVE gets 2x packed modes.
  - measured HW overhead is dominated by per-instruction dispatch (~150ns
    each on top of the cost model), so the kernel is built to MINIMIZE
    INSTRUCTION COUNT: single 256-wide batch chain, merged sigmoid over
    all four gate chunks, x_score as 31 N=256 matmuls in the transposed
    layout with a matmul-based partition sum + gpsimd partition_broadcast
    for the softmax normalization (no per-subtile loops, no transposes),
    broadcast-AP elementwise ops for the scaled identities and the
    attention-weighted input.
  - LSTM state kept as cc = 2*c so tanh comes from the same single
    sigmoid pass (g gate pre-scaled 2x in the weights).
  - next group's bias/W_ih matmuls are emitted after the current step's
    W_hh + sigmoid so the PE works ahead on them during the pointwise
    latency window.

Sharding: data-parallel over batch, 8 cores x 256 rows, weights replicated.
"""

import numpy as np

T = 31          # time steps (T_ref - 1)
D = 128         # input feature dim
H = 128         # hidden dim
G = 4 * H       # gate rows
NCORES = 8
B = 2048
BL = B // NCORES  # 256 batch rows per core

_CACHE = {}


def _build_program(loop_n=0):
    from contextlib import ExitStack

    import concourse.bacc as bacc
    import concourse.mybir as mybir
    import concourse.tile as tile

    f16 = mybir.dt.float16
    f32 = mybir.dt.float32

    nc = bacc.Bacc("TRN2", target_bir_lowering=False, debug=False)

    xt_d = nc.dram_tensor("xt", [D, T, BL], f16, kind="ExternalInput").ap()
    wxb_d = nc.dram_tensor("wxb", [D, T], f16, kind="ExternalInput").ap()
    wih_d = nc.dram_tensor("wih", [D, G], f16, kind="ExternalInput").ap()
    whh_d = nc.dram_tensor("whh", [H, G], f16, kind="ExternalInput").ap()
    bias_d = nc.dram_tensor("bias", [2, G], f16, kind="ExternalInput").ap()
    ident_d = nc.dram_tensor("ident", [D, D], f16, kind="ExternalInput").ap()
    ones_d = nc.dram_tensor("ones", [2, 2 * BL], f16, kind="ExternalInput").ap()

    # device-native output layouts: every output DMA is contiguous on both
    # the SBUF and DRAM side (strided-DRAM-dest DMA is drastically slower
    # on real HW than the cost model suggests); host undoes the layout.
    wt_d = nc.dram_tensor("wt_out", [D, T, BL], f16, kind="ExternalOutput").ap()
    enc_d = nc.dram_tensor("enc_out", [H, T, BL], f16, kind="ExternalOutput").ap()

    with ExitStack() as ctx:
        tc = ctx.enter_context(tile.TileContext(nc))

        def body():
            _emit(nc, tc, ctx, mybir, f16, f32,
                  xt_d, wxb_d, wih_d, whh_d, bias_d, ident_d, ones_d,
                  wt_d, enc_d)

        if loop_n:
            with tc.For_i(0, loop_n, 1):
                body()
        else:
            body()

    nc.compile()
    return nc


def _emit(nc, tc, ctx, mybir, f16, f32,
          xt_d, wxb_d, wih_d, whh_d, bias_d, ident_d, ones_d, wt_d, enc_d):
    from contextlib import ExitStack

    AF = mybir.ActivationFunctionType

    big = ctx.enter_context(tc.tile_pool(name="big", bufs=1))

    # ---- persistent SBUF tensors ----
    xt_s = big.tile([D, T * BL], f16, tag="xt")
    wxt_s = big.tile([D, T * BL], f16, tag="wxt")
    wid_s = big.tile([D, T * D], f16, tag="wid")
    wxb_s = big.tile([D, T], f16, tag="wxb")
    wih_s = big.tile([D, G], f16, tag="wih")
    whh_s = big.tile([H, G], f16, tag="whh")
    bias_s = big.tile([2, G], f16, tag="bias")
    ident_s = big.tile([D, D], f16, tag="ident")
    ones_s = big.tile([2, 2 * BL], f16, tag="ones")
    zro_s = big.tile([H, BL], f16, tag="zro")
    attnT = big.tile([D, BL], f16, tag="attnT")

    nc.sync.dma_start(out=ident_s[:], in_=ident_d[:])
    nc.sync.dma_start(out=wxb_s[:], in_=wxb_d[:])
    # wid[:, t, :] = w_x[t] * I in ONE broadcast-AP op
    nc.vector.tensor_mul(
        wid_s[:].rearrange("p (t j) -> p t j", t=T),
        ident_s[:].unsqueeze(1).broadcast_to([D, T, D]),
        wxb_s[:].unsqueeze(2).broadcast_to([D, T, D]),
    )
    nc.vector.memset(zro_s[:], 0.0)

    for t0 in range(0, T, 16):
        t1 = min(t0 + 16, T)
        nc.sync.dma_start(
            out=xt_s[:, t0 * BL:t1 * BL], in_=xt_d[:, t0:t1, :])
    nc.sync.dma_start(out=wih_s[:], in_=wih_d[:])
    nc.sync.dma_start(out=whh_s[:], in_=whh_d[:])
    nc.sync.dma_start(out=bias_s[:], in_=bias_d[:])
    nc.sync.dma_start(out=ones_s[:], in_=ones_d[:])

    with ExitStack() as fctx:
        frs = fctx.enter_context(tc.tile_pool(name="fsmall", bufs=2))
        psf = fctx.enter_context(tc.tile_pool(name="psf", bufs=2, space="PSUM"))

        # ---- PE warmup: long dummy matmuls keep the clock ramping while
        # the x DMA lands (output discarded).
        pwm = psf.tile([D, 4 * D], f32, tag="warm")
        for w in range(14):
            nc.tensor.matmul(
                pwm[:], lhsT=ident_s[:],
                rhs=ident_s[:].unsqueeze(1).broadcast_to([D, 4, D]),
                start=True, stop=True, skip_group_check=True,
            )

        # ---- x_score in transposed layout: xsT[d, b] += wid_t.T @ xt_t
        ps_xs = psf.tile([D, BL], f32, tag="xs")
        for t in range(T):
            nc.tensor.matmul(
                ps_xs[:],
                lhsT=wid_s[:, t * D:(t + 1) * D],
                rhs=xt_s[:, t * BL:(t + 1) * BL],
                start=(t == 0),
                stop=(t == T - 1),
                skip_group_check=True,
            )

        # ---- softmax over d (the partition axis), max-free: |score| <~ 3
        # exp -> matmul-sum over partitions -> reciprocal -> partition
        # broadcast -> normalize.  attn = exp(s) / sum_d exp(s)
        exps = frs.tile([D, BL], f16, tag="exps")
        nc.scalar.activation(exps[:], ps_xs[:], AF.Exp)
        one1 = frs.tile([D, 1], f16, tag="one1")
        nc.vector.memset(one1[:], 1.0)
        zsum = psf.tile([1, BL], f32, tag="zsum")
        nc.tensor.matmul(zsum[:], lhsT=one1[:], rhs=exps[:],
                         start=True, stop=True, skip_group_check=True)
        rz = frs.tile([1, BL], f32, tag="rz")
        nc.vector.reciprocal(rz[:], zsum[:])
        rzb = frs.tile([D, BL], f32, tag="rzb")
        nc.gpsimd.partition_broadcast(rzb[:], rz[:])
        nc.vector.tensor_mul(attnT[:], exps[:], rzb[:])

        # pre-trigger the Sigmoid table-set load while the front finishes
        sdum = frs.tile([1, 1], f32, tag="sdum")
        nc.scalar.activation(sdum[:], rz[:, 0:1], AF.Sigmoid)

    # ---- LSTM recurrence (single 256-wide chain) ----
    # PSUM ping-pong tiles [128, 2048] (4 banks); bank c = gate chunk c
    # (pytorch order i,f,g,o; g pre-scaled 2x), cols = [t | t+1].
    psg = ctx.enter_context(tc.tile_pool(name="psg", bufs=2, space="PSUM"))
    sgp = ctx.enter_context(tc.tile_pool(name="sg", bufs=4))
    sm = ctx.enter_context(tc.tile_pool(name="small", bufs=6))
    ccp = ctx.enter_context(tc.tile_pool(name="ccp", bufs=4))
    hst = ctx.enter_context(tc.tile_pool(name="hstage", bufs=3))
    jk = ctx.enter_context(tc.tile_pool(name="junk", bufs=4))

    def emit_group_mms(ps, tg):
        """bias + W_ih matmuls for the 2-step group starting at tg."""
        gw = min(2, T - tg)
        nw = gw * BL
        for c in range(4):
            gseg = slice(c * H, (c + 1) * H)
            nc.tensor.matmul(
                ps[:, c * 512:c * 512 + nw], lhsT=bias_s[0:2, gseg],
                rhs=ones_s[0:2, 0:nw], start=True, stop=False,
                skip_group_check=True,
            )
            nc.tensor.matmul(
                ps[:, c * 512:c * 512 + nw], lhsT=wih_s[:, gseg],
                rhs=wxt_s[:, tg * BL:tg * BL + nw], start=False, stop=False,
                skip_group_check=True,
            )

    def emit_wt(tg):
        """weighted input for a 2-step group in one broadcast-AP op."""
        gw = min(2, T - tg)
        nc.vector.tensor_mul(
            wxt_s[:, tg * BL:(tg + gw) * BL].rearrange(
                "d (t b) -> d t b", t=gw),
            xt_s[:, tg * BL:(tg + gw) * BL].rearrange(
                "d (t b) -> d t b", t=gw),
            attnT[:].unsqueeze(1).broadcast_to([D, gw, BL]),
        )

    c_prev = zro_s
    h_prev = zro_s
    hstage = None
    ps_cur = None
    ps_next = None

    # group 0 prep happens up front (overlaps softmax tail / table load)
    emit_wt(0)
    ps_cur = psg.tile([128, 4 * 512], f32, tag="gates", name="ps_g0")
    emit_group_mms(ps_cur, 0)

    for tg in range(0, T, 2):
        g = tg // 2
        gw = min(2, T - tg)
        if tg + 2 < T:
            emit_wt(tg + 2)
        if tg % 8 == 6 or tg + gw == T:  # flush wt_out every 8 steps
            t0 = (tg // 8) * 8
            t1 = min(t0 + 8, T)
            nc.sync.dma_start(
                out=wt_d[:, t0:t1, :],
                in_=wxt_s[:, t0 * BL:t1 * BL].rearrange(
                    "d (t b) -> d t b", b=BL),
            )
        ps_next = None
        for dtw in range(gw):
            t = tg + dtw
            if t % 4 == 0:
                hstage = hst.tile([H, 4 * BL], f16, tag="hst")
            for c in range(4):
                nc.tensor.matmul(
                    ps_cur[:, c * 512 + dtw * BL:c * 512 + (dtw + 1) * BL],
                    lhsT=whh_s[:, c * H:(c + 1) * H],
                    rhs=h_prev[:],
                    start=False, stop=(dtw == gw - 1),
                    skip_group_check=True,
                )
            sg = sgp.tile([128, 4 * BL], f16, tag="sg", name=f"sg_{t}")
            ps_slot = ps_cur[:].rearrange("p (c x) -> p c x", c=4)[
                :, :, dtw * BL:(dtw + 1) * BL]
            nc.scalar.activation(sg[:], ps_slot, AF.Sigmoid)
            # next group's bias/W_ih matmuls: PE works ahead on these
            # during the sigmoid/pointwise latency window.
            if tg + 2 < T and dtw == 0:
                ps_next = psg.tile([128, 4 * 512], f32, tag="gates",
                                   name=f"ps_g{g + 1}")
                emit_group_mms(ps_next, tg + 2)

            si = sg[:, 0 * BL:1 * BL]
            sf = sg[:, 1 * BL:2 * BL]
            s2g = sg[:, 2 * BL:3 * BL]
            so = sg[:, 3 * BL:4 * BL]

            t1_ = sm.tile([H, BL], f16, tag="t1", name=f"t1_{t}")
            nc.gpsimd.tensor_mul(t1_[:], sf, c_prev[:])
            t2 = sm.tile([H, BL], f16, tag="t2", name=f"t2_{t}")
            j1 = jk.tile([H, 1], f32, tag="j1", name=f"j1_{t}")
            # t2 = sigmoid(i)*tanh(g) = (2*s2g - 1) * si
            nc.vector.affine_mul_reduce(
                out=t2[:], accum_out=j1[:], in0=s2g, in1=si,
                scale=2.0, bias=-1.0,
            )
            cc = ccp.tile([H, BL], f16, tag="c", name=f"c_{t}")
            nc.vector.tensor_add(cc[:], t1_[:], t2[:])
            # tanh and sigmoid live in the same ACT table set, so Tanh here
            # costs no table switch and h becomes a fast packed multiply.
            tc = sm.tile([H, BL], f16, tag="s2c", name=f"s2c_{t}")
            nc.scalar.activation(tc[:], cc[:], AF.Tanh)
            h_new = hstage[:, (t % 4) * BL:(t % 4 + 1) * BL]
            nc.vector.tensor_mul(h_new, tc[:], so)
            c_prev = cc
            h_prev = _Slice(h_new)
            if t % 4 == 3 or t == T - 1:
                t0 = (t // 4) * 4
                n = t - t0 + 1
                nc.sync.dma_start(
                    out=enc_d[:, t0:t0 + n, :],
                    in_=hstage[:].rearrange("h (t b) -> h t b", t=4)[:, :n, :],
                )
        ps_cur = ps_next


class _Slice:
    """Tiny adapter so h_prev[:] works for both tiles and AP slices."""

    def __init__(self, ap):
        self._ap = ap

    def __getitem__(self, key):
        return self._ap


def _get_program():
    if "nc" not in _CACHE:
        _CACHE["nc"] = _build_program()
    return _CACHE["nc"]


def _host_inputs(input_data, W_ih, W_hh, b_ih, b_hh, attn_w, attn_b):
    """Build the per-core input maps (host-side prep is weights-only +
    layout transforms + fp16 casts)."""
    x = np.asarray(input_data, dtype=np.float32)
    W_ih = np.asarray(W_ih, dtype=np.float32)
    W_hh = np.asarray(W_hh, dtype=np.float32)
    b = (np.asarray(b_ih, dtype=np.float32)
         + np.asarray(b_hh, dtype=np.float32))
    w_x = np.asarray(attn_w, dtype=np.float32)[2 * H:]  # only the x part

    # scale the g-gate block (pytorch order i,f,g,o -> rows 2H:3H) by 2
    # so tanh(g) = 2*sigmoid(2g) - 1 works with a single sigmoid pass.
    scale = np.ones((G, 1), np.float32)
    scale[2 * H:3 * H] = 2.0
    wih_t = np.ascontiguousarray((W_ih * scale).T).astype(np.float16)
    whh_t = np.ascontiguousarray((W_hh * scale).T).astype(np.float16)
    bm = (b[None, :] * scale.T).astype(np.float32)
    b_hi = bm.astype(np.float16)
    b_lo = (bm - b_hi.astype(np.float32)).astype(np.float16)
    bias_m = np.ascontiguousarray(np.concatenate([b_hi, b_lo], 0))  # [2, 4H]

    wxb = np.ascontiguousarray(
        np.tile(w_x[None, :], (D, 1))).astype(np.float16)  # [D, T]
    ident = np.eye(D, dtype=np.float16)
    ones = np.ones((2, 2 * BL), np.float16)

    in_maps = []
    for i in range(NCORES):
        xs = x[i * BL:(i + 1) * BL]                  # [BL, T, D]
        xt = np.ascontiguousarray(
            xs.transpose(2, 1, 0)).astype(np.float16)  # [D, T, BL]
        in_maps.append({
            "xt": xt,
            "wxb": wxb,
            "wih": wih_t,
            "whh": whh_t,
            "bias": bias_m,
            "ident": ident,
            "ones": ones,
        })
    return in_maps


def _gather(results):
    weighted = np.empty((B, T, D), np.float32)
    encoded = np.empty((B, T, H), np.float32)
    for i, r in enumerate(results):
        # wt_out/enc_out are fp16 [D|H, T, BL] -> [BL, T, D|H] fp32
        weighted[i * BL:(i + 1) * BL] = \
            r["wt_out"].transpose(2, 1, 0).astype(np.float32)
        encoded[i * BL:(i + 1) * BL] = \
            r["enc_out"].transpose(2, 1, 0).astype(np.float32)
    return weighted, encoded


def kernel(input_data, W_ih, W_hh, b_ih, b_hh, attn_w, attn_b):
    from concourse.bass_utils import run_bass_kernel_spmd

    nc = _get_program()
    in_maps = _host_inputs(input_data, W_ih, W_hh, b_ih, b_hh, attn_w, attn_b)
    res = run_bass_kernel_spmd(nc, in_maps, list(range(NCORES)))
    return _gather(res.results)


# revision 19
# speedup vs baseline: 1.2644x; 1.0253x over previous
"""DA-RNN style encoder (LSTM + input attention) on 8 Trainium2 cores.

Problem: nn_Encoder_63024350101963
  B=2048, T-1=31 steps, D=128 input feats, H=128 hidden.

Key algebraic fact exploited: in the reference,
    score = (h @ w_h + c @ w_c + b)[:, None] + x_score
the recurrent term is constant along the softmax axis, and softmax is
shift-invariant, so
    attn = softmax(x_score)      (time-constant, recurrence-independent)
Therefore weighted = attn[:,None,:] * x  is a pure elementwise op and only
the LSTM cell recurrence is serial.

Design notes (v3):
  - fp16 end-to-end (tolerance gate 2e-2): halves DMA bytes, all matmuls
    run at 1 cycle/row, DVE gets 2x packed modes.
  - measured HW overhead is dominated by per-instruction dispatch (~150ns
    each on top of the cost model), so the kernel is built to MINIMIZE
    INSTRUCTION COUNT: single 256-wide batch chain, merged sigmoid over
    all four gate chunks, x_score as 31 N=256 matmuls in the transposed
    layout with a matmul-based partition sum + gpsimd partition_broadcast
    for the softmax normalization (no per-subtile loops, no transposes),
    broadcast-AP elementwise ops for the scaled identities and the
    attention-weighted input.
  - LSTM state kept as cc = 2*c so tanh comes from the same single
    sigmoid pass (g gate pre-scaled 2x in the weights).
  - next group's bias/W_ih matmuls are emitted after the current step's
    W_hh + sigmoid so the PE works ahead on them during the pointwise
    latency window.

Sharding: data-parallel over batch, 8 cores x 256 rows, weights replicated.
"""

import numpy as np

T = 31          # time steps (T_ref - 1)
D = 128         # input feature dim
H = 128         # hidden dim
G = 4 * H       # gate rows
NCORES = 8
B = 2048
BL = B // NCORES  # 256 batch rows per core

_CACHE = {}


def _build_program(loop_n=0):
    from contextlib import ExitStack

    import concourse.bacc as bacc
    import concourse.mybir as mybir
    import concourse.tile as tile

    f16 = mybir.dt.float16
    f32 = mybir.dt.float32

    nc = bacc.Bacc("TRN2", target_bir_lowering=False, debug=False)

    xt_d = nc.dram_tensor("xt", [D, T, BL], f16, kind="ExternalInput").ap()
    wxb_d = nc.dram_tensor("wxb", [D, T], f16, kind="ExternalInput").ap()
    wih_d = nc.dram_tensor("wih", [D, G], f16, kind="ExternalInput").ap()
    whh_d = nc.dram_tensor("whh", [H, G], f16, kind="ExternalInput").ap()
    bias_d = nc.dram_tensor("bias", [2, G], f16, kind="ExternalInput").ap()
    ident_d = nc.dram_tensor("ident", [D, D], f16, kind="ExternalInput").ap()
    ones_d = nc.dram_tensor("ones", [2, 2 * BL], f16, kind="ExternalInput").ap()

    # device-native output layouts: every output DMA is contiguous on both
    # the SBUF and DRAM side (strided-DRAM-dest DMA is drastically slower
    # on real HW than the cost model suggests); host undoes the layout.
    wt_d = nc.dram_tensor("wt_out", [D, T, BL], f16, kind="ExternalOutput").ap()
    enc_d = nc.dram_tensor("enc_out", [H, T, BL], f16, kind="ExternalOutput").ap()

    with ExitStack() as ctx:
        tc = ctx.enter_context(tile.TileContext(nc))

        def body():
            _emit(nc, tc, ctx, mybir, f16, f32,
                  xt_d, wxb_d, wih_d, whh_d, bias_d, ident_d, ones_d,
                  wt_d, enc_d)

        if loop_n:
            with tc.For_i(0, loop_n, 1):
                body()
        else:
            body()

    nc.compile()
    return nc


def _emit(nc, tc, ctx, mybir, f16, f32,
          xt_d, wxb_d, wih_d, whh_d, bias_d, ident_d, ones_d, wt_d, enc_d):
    from contextlib import ExitStack

    AF = mybir.ActivationFunctionType

    big = ctx.enter_context(tc.tile_pool(name="big", bufs=1))

    # ---- persistent SBUF tensors ----
    xt_s = big.tile([D, T * BL], f16, tag="xt")
    wxt_s = big.tile([D, T * BL], f16, tag="wxt")
    wid_s = big.tile([D, T * D], f16, tag="wid")
    wxb_s = big.tile([D, T], f16, tag="wxb")
    wih_s = big.tile([D, G], f16, tag="wih")
    whh_s = big.tile([H, G], f16, tag="whh")
    bias_s = big.tile([2, G], f16, tag="bias")
    ident_s = big.tile([D, D], f16, tag="ident")
    ones_s = big.tile([2, 2 * BL], f16, tag="ones")
    zro_s = big.tile([H, BL], f16, tag="zro")
    attnT = big.tile([D, BL], f16, tag="attnT")

    nc.sync.dma_start(out=ident_s[:], in_=ident_d[:])
    nc.sync.dma_start(out=wxb_s[:], in_=wxb_d[:])
    # wid[:, t, :] = w_x[t] * I in ONE broadcast-AP op
    nc.vector.tensor_mul(
        wid_s[:].rearrange("p (t j) -> p t j", t=T),
        ident_s[:].unsqueeze(1).broadcast_to([D, T, D]),
        wxb_s[:].unsqueeze(2).broadcast_to([D, T, D]),
    )
    nc.vector.memset(zro_s[:], 0.0)

    for t0 in range(0, T, 16):
        t1 = min(t0 + 16, T)
        nc.sync.dma_start(
            out=xt_s[:, t0 * BL:t1 * BL], in_=xt_d[:, t0:t1, :])
    nc.sync.dma_start(out=wih_s[:], in_=wih_d[:])
    nc.sync.dma_start(out=whh_s[:], in_=whh_d[:])
    nc.sync.dma_start(out=bias_s[:], in_=bias_d[:])
    nc.sync.dma_start(out=ones_s[:], in_=ones_d[:])

    with ExitStack() as fctx:
        frs = fctx.enter_context(tc.tile_pool(name="fsmall", bufs=2))
        psf = fctx.enter_context(tc.tile_pool(name="psf", bufs=2, space="PSUM"))

        # ---- PE warmup: long dummy matmuls keep the clock ramping while
        # the x DMA lands (output discarded).
        pwm = psf.tile([D, 4 * D], f32, tag="warm")
        for w in range(14):
            nc.tensor.matmul(
                pwm[:], lhsT=ident_s[:],
                rhs=ident_s[:].unsqueeze(1).broadcast_to([D, 4, D]),
                start=True, stop=True, skip_group_check=True,
            )

        # ---- x_score in transposed layout: xsT[d, b] += wid_t.T @ xt_t
        ps_xs = psf.tile([D, BL], f32, tag="xs")
        for t in range(T):
            nc.tensor.matmul(
                ps_xs[:],
                lhsT=wid_s[:, t * D:(t + 1) * D],
                rhs=xt_s[:, t * BL:(t + 1) * BL],
                start=(t == 0),
                stop=(t == T - 1),
                skip_group_check=True,
            )

        # ---- softmax over d (the partition axis), max-free: |score| <~ 3
        # exp -> matmul-sum over partitions -> reciprocal -> partition
        # broadcast -> normalize.  attn = exp(s) / sum_d exp(s)
        exps = frs.tile([D, BL], f16, tag="exps")
        nc.scalar.activation(exps[:], ps_xs[:], AF.Exp)
        one1 = frs.tile([D, 1], f16, tag="one1")
        nc.vector.memset(one1[:], 1.0)
        zsum = psf.tile([1, BL], f32, tag="zsum")
        nc.tensor.matmul(zsum[:], lhsT=one1[:], rhs=exps[:],
                         start=True, stop=True, skip_group_check=True)
        rz = frs.tile([1, BL], f32, tag="rz")
        nc.vector.reciprocal(rz[:], zsum[:])
        rzb = frs.tile([D, BL], f32, tag="rzb")
        nc.gpsimd.partition_broadcast(rzb[:], rz[:])
        nc.vector.tensor_mul(attnT[:], exps[:], rzb[:])

        # pre-trigger the Sigmoid table-set load while the front finishes
        sdum = frs.tile([1, 1], f32, tag="sdum")
        nc.scalar.activation(sdum[:], rz[:, 0:1], AF.Sigmoid)

    # ---- LSTM recurrence (single 256-wide chain) ----
    # PSUM ping-pong tiles [128, 2048] (4 banks); bank c = gate chunk c
    # (pytorch order i,f,g,o; g pre-scaled 2x), cols = [t | t+1].
    psg = ctx.enter_context(tc.tile_pool(name="psg", bufs=2, space="PSUM"))
    sgp = ctx.enter_context(tc.tile_pool(name="sg", bufs=4))
    sm = ctx.enter_context(tc.tile_pool(name="small", bufs=6))
    ccp = ctx.enter_context(tc.tile_pool(name="ccp", bufs=4))
    hst = ctx.enter_context(tc.tile_pool(name="hstage", bufs=3))
    jk = ctx.enter_context(tc.tile_pool(name="junk", bufs=4))

    def emit_group_mms(ps, tg):
        """bias + W_ih matmuls for the 2-step group starting at tg."""
        gw = min(2, T - tg)
        nw = gw * BL
        for c in range(4):
            gseg = slice(c * H, (c + 1) * H)
            nc.tensor.matmul(
                ps[:, c * 512:c * 512 + nw], lhsT=bias_s[0:2, gseg],
                rhs=ones_s[0:2, 0:nw], start=True, stop=False,
                skip_group_check=True,
            )
            nc.tensor.matmul(
                ps[:, c * 512:c * 512 + nw], lhsT=wih_s[:, gseg],
                rhs=wxt_s[:, tg * BL:tg * BL + nw], start=False, stop=False,
                skip_group_check=True,
            )

    def emit_wt(tg):
        """weighted input for a 2-step group in one broadcast-AP op."""
        gw = min(2, T - tg)
        nc.vector.tensor_mul(
            wxt_s[:, tg * BL:(tg + gw) * BL].rearrange(
                "d (t b) -> d t b", t=gw),
            xt_s[:, tg * BL:(tg + gw) * BL].rearrange(
                "d (t b) -> d t b", t=gw),
            attnT[:].unsqueeze(1).broadcast_to([D, gw, BL]),
        )

    c_prev = zro_s
    h_prev = zro_s
    hstage = None
    ps_cur = None
    ps_next = None

    # group 0 prep happens up front (overlaps softmax tail / table load)
    emit_wt(0)
    ps_cur = psg.tile([128, 4 * 512], f32, tag="gates", name="ps_g0")
    emit_group_mms(ps_cur, 0)

    for tg in range(0, T, 2):
        g = tg // 2
        gw = min(2, T - tg)
        if tg + 2 < T:
            emit_wt(tg + 2)
        if tg % 8 == 6 or tg + gw == T:  # flush wt_out every 8 steps
            t0 = (tg // 8) * 8
            t1 = min(t0 + 8, T)
            nc.sync.dma_start(
                out=wt_d[:, t0:t1, :],
                in_=wxt_s[:, t0 * BL:t1 * BL].rearrange(
                    "d (t b) -> d t b", b=BL),
            )
        ps_next = None
        for dtw in range(gw):
            t = tg + dtw
            if t % 8 == 0:
                hstage = hst.tile([H, 8 * BL], f16, tag="hst")
            for c in range(4):
                nc.tensor.matmul(
                    ps_cur[:, c * 512 + dtw * BL:c * 512 + (dtw + 1) * BL],
                    lhsT=whh_s[:, c * H:(c + 1) * H],
                    rhs=h_prev[:],
                    start=False, stop=(dtw == gw - 1),
                    skip_group_check=True,
                )
            sg = sgp.tile([128, 4 * BL], f16, tag="sg", name=f"sg_{t}")
            ps_slot = ps_cur[:].rearrange("p (c x) -> p c x", c=4)[
                :, :, dtw * BL:(dtw + 1) * BL]
            nc.scalar.activation(sg[:], ps_slot, AF.Sigmoid)
            # next group's bias/W_ih matmuls: PE works ahead on these
            # during the sigmoid/pointwise latency window.
            if tg + 2 < T and dtw == 0:
                ps_next = psg.tile([128, 4 * 512], f32, tag="gates",
                                   name=f"ps_g{g + 1}")
                emit_group_mms(ps_next, tg + 2)

            si = sg[:, 0 * BL:1 * BL]
            sf = sg[:, 1 * BL:2 * BL]
            s2g = sg[:, 2 * BL:3 * BL]
            so = sg[:, 3 * BL:4 * BL]

            t1_ = sm.tile([H, BL], f16, tag="t1", name=f"t1_{t}")
            nc.gpsimd.tensor_mul(t1_[:], sf, c_prev[:])
            t2 = sm.tile([H, BL], f16, tag="t2", name=f"t2_{t}")
            j1 = jk.tile([H, 1], f32, tag="j1", name=f"j1_{t}")
            # t2 = sigmoid(i)*tanh(g) = (2*s2g - 1) * si
            nc.vector.affine_mul_reduce(
                out=t2[:], accum_out=j1[:], in0=s2g, in1=si,
                scale=2.0, bias=-1.0,
            )
            cc = ccp.tile([H, BL], f16, tag="c", name=f"c_{t}")
            nc.vector.tensor_add(cc[:], t1_[:], t2[:])
            # tanh and sigmoid live in the same ACT table set, so Tanh here
            # costs no table switch and h becomes a fast packed multiply.
            tc = sm.tile([H, BL], f16, tag="s2c", name=f"s2c_{t}")
            nc.scalar.activation(tc[:], cc[:], AF.Tanh)
            h_new = hstage[:, (t % 8) * BL:(t % 8 + 1) * BL]
            nc.vector.tensor_mul(h_new, tc[:], so)
            c_prev = cc
            h_prev = _Slice(h_new)
            if t % 8 == 7 or t == T - 1:
                t0 = (t // 8) * 8
                n = t - t0 + 1
                nc.sync.dma_start(
                    out=enc_d[:, t0:t0 + n, :],
                    in_=hstage[:].rearrange("h (t b) -> h t b", t=8)[:, :n, :],
                )
        ps_cur = ps_next


class _Slice:
    """Tiny adapter so h_prev[:] works for both tiles and AP slices."""

    def __init__(self, ap):
        self._ap = ap

    def __getitem__(self, key):
        return self._ap


def _get_program():
    if "nc" not in _CACHE:
        _CACHE["nc"] = _build_program()
    return _CACHE["nc"]


def _host_inputs(input_data, W_ih, W_hh, b_ih, b_hh, attn_w, attn_b):
    """Build the per-core input maps (host-side prep is weights-only +
    layout transforms + fp16 casts)."""
    x = np.asarray(input_data, dtype=np.float32)
    W_ih = np.asarray(W_ih, dtype=np.float32)
    W_hh = np.asarray(W_hh, dtype=np.float32)
    b = (np.asarray(b_ih, dtype=np.float32)
         + np.asarray(b_hh, dtype=np.float32))
    w_x = np.asarray(attn_w, dtype=np.float32)[2 * H:]  # only the x part

    # scale the g-gate block (pytorch order i,f,g,o -> rows 2H:3H) by 2
    # so tanh(g) = 2*sigmoid(2g) - 1 works with a single sigmoid pass.
    scale = np.ones((G, 1), np.float32)
    scale[2 * H:3 * H] = 2.0
    wih_t = np.ascontiguousarray((W_ih * scale).T).astype(np.float16)
    whh_t = np.ascontiguousarray((W_hh * scale).T).astype(np.float16)
    bm = (b[None, :] * scale.T).astype(np.float32)
    b_hi = bm.astype(np.float16)
    b_lo = (bm - b_hi.astype(np.float32)).astype(np.float16)
    bias_m = np.ascontiguousarray(np.concatenate([b_hi, b_lo], 0))  # [2, 4H]

    wxb = np.ascontiguousarray(
        np.tile(w_x[None, :], (D, 1))).astype(np.float16)  # [D, T]
    ident = np.eye(D, dtype=np.float16)
    ones = np.ones((2, 2 * BL), np.float16)

    in_maps = []
    for i in range(NCORES):
        xs = x[i * BL:(i + 1) * BL]                  # [BL, T, D]
        xt = np.ascontiguousarray(
            xs.transpose(2, 1, 0)).astype(np.float16)  # [D, T, BL]
        in_maps.append({
            "xt": xt,
            "wxb": wxb,
            "wih": wih_t,
            "whh": whh_t,
            "bias": bias_m,
            "ident": ident,
            "ones": ones,
        })
    return in_maps


def _gather(results):
    weighted = np.empty((B, T, D), np.float32)
    encoded = np.empty((B, T, H), np.float32)
    for i, r in enumerate(results):
        # wt_out/enc_out are fp16 [D|H, T, BL] -> [BL, T, D|H] fp32
        weighted[i * BL:(i + 1) * BL] = \
            r["wt_out"].transpose(2, 1, 0).astype(np.float32)
        encoded[i * BL:(i + 1) * BL] = \
            r["enc_out"].transpose(2, 1, 0).astype(np.float32)
    return weighted, encoded


def kernel(input_data, W_ih, W_hh, b_ih, b_hh, attn_w, attn_b):
    from concourse.bass_utils import run_bass_kernel_spmd

    nc = _get_program()
    in_maps = _host_inputs(input_data, W_ih, W_hh, b_ih, b_hh, attn_w, attn_b)
    res = run_bass_kernel_spmd(nc, in_maps, list(range(NCORES)))
    return _gather(res.results)
